# revision 1
# baseline (speedup 1.0000x reference)
"""CrissCrossAttention Trainium2 kernel (8 NeuronCores, data-parallel).

Problem: B=4, C=256, H=W=128, 4 heads. Per head: cq=8 q/k channels, cv=64
v channels. Row attention (over W per row) + column attention (over H per
column), outputs added with the CCNet spatial-transpose quirk, then
out = gamma*attn + x.

Sharding: 16 (batch, head) pairs over 8 cores -> each core handles
batch b = core//2 and head pair p = core%2 (global heads 2p, 2p+1).
Each core reads x[b] (all 256 channels, needed by the projections) and
produces output channels [128p : 128p+128] of batch b.

Core-local pipeline (pixels indexed pix = h*128 + w):
  - qk projection -> flat row-major fr[32, h*128+w] and col-major
    fc[32, w*128+h] bf16 stores. fc comes from a second matmul pass with a
    column-ordered (strided) moving operand so both evacuations write
    near-contiguously. Bias is fused into the PSUM evacuation.
  - band-packed operand stores for the PE (matmul operands must start at
    32-aligned partitions): q/k value for row h lives at partition
    32*(h%4)+c -> the 4 rows of a group occupy distinct PE row-groups and
    their K=8 energy matmuls run concurrently via tile_position (each into
    its own PSUM bank -- concurrent row-group matmuls must not share one).
    Built from the flat stores with SBUF->SBUF DMAs (off-engine).
  - vT projection (pixel-major): vT[128w, 128h, 128c] bf16, channels =
    [64 head0 | 64 head1], Wv and bv pre-scaled by gamma on host.
  - vTc[h, w, c] = spatial transpose of vT via per-channel PE transposes
    (the DMA xbar path is a single ~26 GB/s unit -- 160 us serial stall).
  - Per row r, head hh:  eT[v,w] = k^T q (PE, K=8, 4 rows concurrent);
    pT = exp(eT) (ACT, no max subtraction -- logits are O(10));
    o2[w, 0:64] = pT.T @ vT slice; o2[w,64] = colsum via ones column
    matmul reusing the same stationary pT (softmax denominator);
    t[w, c] = o2[:, 0:64] * recip(o2[:, 64]) (DVE).
  - Column attention identical using qc/kc stores and vTc. The CCNet
    transpose aligns row-tile(row i) and col-tile(col i) elementwise on
    output row i: attn_un[j, c] = t_row(i)[j,c] + t_col(i)[j,c] (GpSimd).
  - PE-transpose attn_un (bf16) to channel-major, add residual x, DMA out.
"""

import os
import numpy as np
from contextlib import ExitStack

import concourse.bass as bass
import concourse.bacc as bacc
import concourse.tile as tile
from concourse import mybir
from concourse.masks import make_identity

F32 = mybir.dt.float32
BF16 = mybir.dt.bfloat16

B, C, H, W = 4, 256, 128, 128
PIX = H * W            # 16384
CV = 64                # v channels per head
NCORES = 8
G = 4                  # rows per attention group (= PE row-group packing)
NG = H // G            # 32 groups


def build_program():
    nc = bacc.Bacc("TRN2", target_bir_lowering=False, debug=False,
                   num_devices=NCORES)

    x_in = nc.dram_tensor("x_in", [C, PIX], F32, kind="ExternalInput")
    x_res = nc.dram_tensor("x_res", [128, PIX], F32, kind="ExternalInput")
    wqkT = nc.dram_tensor("wqkT", [C, 32], BF16, kind="ExternalInput")
    qk_bias = nc.dram_tensor("qk_bias", [32, 1], F32, kind="ExternalInput")
    wvT = nc.dram_tensor("wvT", [C, 130], BF16, kind="ExternalInput")
    vbias_row = nc.dram_tensor("vbias_row", [1, 130], BF16, kind="ExternalInput")
    out = nc.dram_tensor("out", [128, PIX], F32, kind="ExternalOutput")

    with tile.TileContext(nc) as tc, ExitStack() as ctx:
        consts = ctx.enter_context(tc.tile_pool(name="consts", bufs=1))
        persist = ctx.enter_context(tc.tile_pool(name="persist", bufs=1))

        # constants / weights
        wqa = consts.tile([128, 32], BF16, tag="wqa")
        wqb = consts.tile([128, 32], BF16, tag="wqb")
        nc.sync.dma_start(wqa, wqkT[0:128, :])
        nc.sync.dma_start(wqb, wqkT[128:256, :])
        wva = consts.tile([128, 130], BF16, tag="wva")
        wvb = consts.tile([128, 130], BF16, tag="wvb")
        nc.sync.dma_start(wva, wvT[0:128, :])
        nc.sync.dma_start(wvb, wvT[128:256, :])
        qkb = consts.tile([32, 1], F32, tag="qkb")
        nc.sync.dma_start(qkb, qk_bias[:, :])
        vbias2 = consts.tile([1, 2, 130], BF16, tag="vbias2")
        nc.sync.dma_start(vbias2[:, 0, :], vbias_row[:, :])
        nc.sync.dma_start(vbias2[:, 1, :], vbias_row[:, :])
        ones1 = consts.tile([1, 128], BF16, tag="ones1")
        nc.vector.memset(ones1, 1.0)
        identb = consts.tile([128, 128], BF16, tag="identb")
        make_identity(nc, identb)

        # persistent activations
        # band-packed operand stores: partition 32*(h%4)+c, c<8
        q_sb = persist.tile([128, 2, H // 4, W], BF16, tag="q")    # 16 KiB
        k_sb = persist.tile([128, 2, H // 4, W], BF16, tag="k")    # 16 KiB
        qc_sb = persist.tile([128, 2, W // 4, H], BF16, tag="qc")  # 16 KiB
        kc_sb = persist.tile([128, 2, W // 4, H], BF16, tag="kc")  # 16 KiB
        # pixel-major value stores, channel innermost
        vT_sb = persist.tile([128, H, 130], BF16, tag="vT")        # 32.5 KiB
        vTc_sb = persist.tile([128, W, 130], BF16, tag="vTc")      # 32.5 KiB

        # ---------------- Phase B: projections ----------------
        with (
            tc.tile_pool(name="qkflat", bufs=1) as flatpool,
            tc.tile_pool(name="xchunk", bufs=2) as xpool,
            tc.tile_pool(name="pq", bufs=2, space="PSUM") as pqpool,
            tc.tile_pool(name="pv", bufs=4, space="PSUM") as pvpool,
        ):
            fr = flatpool.tile([32, PIX], BF16, tag="fr")  # [c, h*128+w]
            fc = flatpool.tile([32, PIX], BF16, tag="fc")  # [c, w*128+h]

            CHUNK = 512  # pixels per chunk = 4 rows
            NCH = PIX // CHUNK
            for chi in range(NCH):
                c0 = chi * CHUNK
                r0 = c0 // 128
                eng = nc.sync if chi % 2 == 0 else nc.scalar
                xa = xpool.tile([128, CHUNK], F32, tag="xa")
                xb = xpool.tile([128, CHUNK], F32, tag="xb")
                eng.dma_start(xa, x_in[0:128, c0 : c0 + CHUNK])
                eng.dma_start(xb, x_in[128:256, c0 : c0 + CHUNK])
                # bf16 copies: cheaper LDWEIGHTS (FWL) for the matmuls
                xab = xpool.tile([128, CHUNK], BF16, tag="xab")
                xbb = xpool.tile([128, CHUNK], BF16, tag="xbb")
                nc.vector.tensor_copy(xab, xa[:, :])
                nc.vector.tensor_copy(xbb, xb[:, :])
                xav = xab[:, :].rearrange("p (r w) -> p r w", w=128)
                xbv = xbb[:, :].rearrange("p (r w) -> p r w", w=128)

                # qk projection, row-pixel order
                pq = pqpool.tile([32, 512], F32, tag="pq")
                nc.tensor.matmul(pq, wqa, xab[:, :], start=True, stop=False)
                nc.tensor.matmul(pq, wqb, xbb[:, :], start=False, stop=True)
                nc.vector.tensor_scalar_add(fr[:, c0 : c0 + CHUNK], pq, qkb)

                # vT projection: 2 rows per PSUM half-bank tile
                for s2 in range(2):
                    pv = pvpool.tile([128, 2, 130], F32, tag="pv")
                    for s3 in range(2):
                        srow = 2 * s2 + s3
                        # start=True only on the bank's first matmul: its
                        # has_written clear is bank-wide, and the shared
                        # bias matmul must still see row0's bits set
                        nc.tensor.matmul(pv[:, s3, :], xav[:, srow, :], wva,
                                         start=(s3 == 0), stop=False,
                                         skip_group_check=True)
                        nc.tensor.matmul(pv[:, s3, :], xbv[:, srow, :], wvb,
                                         start=False, stop=False,
                                         skip_group_check=True)
                    nc.tensor.matmul(pv[:, :, :], ones1, vbias2,
                                     start=False, stop=True,
                                     skip_group_check=True)
                    nc.scalar.copy(
                        vT_sb[:, r0 + 2 * s2 : r0 + 2 * s2 + 2, :], pv)

                # col-major flat store slices: fc[:, :, h-slice] only needs
                # fr rows h-slice -> overlap the permute with projection
                if chi % 8 == 7:
                    hs = (chi // 8) * 32
                    frv = fr[:, :].rearrange("c (h w) -> c w h", w=W)
                    fcv = fc[:, :].rearrange("c (w h) -> c w h", h=H)
                    nc.vector.tensor_copy(fcv[:, :, hs : hs + 32],
                                          frv[:, :, hs : hs + 32])


            # band the flat stores (SBUF->SBUF DMA, partition moves)
            # fr [c, (hb b w)] -> q_sb[32b+c, hh, hb, w]
            for bb in range(4):
                for hh in range(2):
                    src_r = fr[:, :].rearrange(
                        "c (hb b w) -> c b hb w", b=4, w=W)
                    src_c = fc[:, :].rearrange(
                        "c (wb b h) -> c b wb h", b=4, h=H)
                    eng = nc.sync if hh == 0 else nc.scalar
                    eng.dma_start(
                        q_sb[32 * bb : 32 * bb + 8, hh, :, :],
                        src_r[8 * hh : 8 * hh + 8, bb, :, :])
                    eng.dma_start(
                        k_sb[32 * bb : 32 * bb + 8, hh, :, :],
                        src_r[16 + 8 * hh : 24 + 8 * hh, bb, :, :])
                    eng.dma_start(
                        qc_sb[32 * bb : 32 * bb + 8, hh, :, :],
                        src_c[8 * hh : 8 * hh + 8, bb, :, :])
                    eng.dma_start(
                        kc_sb[32 * bb : 32 * bb + 8, hh, :, :],
                        src_c[16 + 8 * hh : 24 + 8 * hh, bb, :, :])

        # ---------------- Phase B2: vTc via PE transposes ----------------
        # vT[w, h, c] -> vTc[h, w, c]; per channel, batched 4 per bank.
        with tc.tile_pool(name="ptr", bufs=2, space="PSUM") as ptrpool:
            for cb in range(33):
                nch = min(4, 130 - cb * 4)
                ptr = ptrpool.tile([128, 4, 128], BF16, tag="ptr")
                for cj in range(nch):
                    cch = cb * 4 + cj
                    nc.tensor.matmul(ptr[:, cj, :], vT_sb[:, :, cch], identb,
                                     start=True, stop=True, is_transpose=True)
                nc.vector.tensor_copy(
                    vTc_sb[:, :, cb * 4 : cb * 4 + nch],
                    ptr[:, 0:nch, :].rearrange("p c w -> p w c"))

        # ---------------- Phase C: attention ----------------
        with (
            tc.tile_pool(name="pe", bufs=1, space="PSUM") as pepool,
            tc.tile_pool(name="po", bufs=2, space="PSUM") as popool,
            tc.tile_pool(name="pat", bufs=2, space="PSUM") as patpool,
            tc.tile_pool(name="pt", bufs=3) as ptpool,
            tc.tile_pool(name="tt", bufs=3) as tpool,
            tc.tile_pool(name="au", bufs=2) as aupool,
            tc.tile_pool(name="rc", bufs=4) as rcpool,
            tc.tile_pool(name="io", bufs=3) as iopool,
        ):
            for g in range(NG):
                t_dir = []
                for d in range(2):  # 0 = row attention, 1 = column attention
                    qs = q_sb if d == 0 else qc_sb
                    ks = k_sb if d == 0 else kc_sb
                    vs = vT_sb if d == 0 else vTc_sb
                    til = tpool.tile([128, G, 2, CV], BF16, tag="t")
                    for hh in range(2):
                        # one PSUM bank per concurrent row-group matmul
                        pe = pepool.tile([128, G, 512], F32, tag="pe")
                        for j in range(G):
                            nc.tensor.matmul(
                                pe[:, j, 0:128],
                                ks[32 * j : 32 * j + 8, hh, g, :],
                                qs[32 * j : 32 * j + 8, hh, g, :],
                                start=True, stop=True,
                                tile_position=(32 * j, 0),
                            )
                        pT = ptpool.tile([128, G, 128], BF16, tag="pt")
                        nc.scalar.activation(pT, pe[:, :, 0:128],
                                             mybir.ActivationFunctionType.Exp)
                        po = popool.tile([128, G, 65], F32, tag="po")
                        for j in range(G):
                            i = g * G + j
                            nc.tensor.matmul(
                                po[:, j, :], pT[:, j, :],
                                vs[:, i, 65 * hh : 65 * hh + 65],
                                start=True, stop=True,
                            )
                        rec = rcpool.tile([128, G, 1], F32, tag="rc")
                        nc.vector.reciprocal(rec, po[:, :, 64:65])
                        nc.vector.tensor_tensor(
                            til[:, :, hh, :], po[:, :, 0:64],
                            rec.to_broadcast((128, G, CV)),
                            mybir.AluOpType.mult,
                        )
                    t_dir.append(til)
                au = aupool.tile([128, G, 128], BF16, tag="au")
                nc.gpsimd.tensor_tensor(au, t_dir[0][:, :, :, :],
                                        t_dir[1][:, :, :, :],
                                        mybir.AluOpType.add)
                pat = patpool.tile([128, G, 128], BF16, tag="pat")
                for j in range(G):
                    nc.tensor.matmul(pat[:, j, :], au[:, j, :], identb,
                                     start=True, stop=True, is_transpose=True)
                eng = nc.sync if g % 2 == 0 else nc.scalar
                xres = iopool.tile([128, G * 128], F32, tag="xres")
                eng.dma_start(xres, x_res[:, g * 512 : (g + 1) * 512])
                res = iopool.tile([128, G * 128], F32, tag="res")
                nc.vector.tensor_tensor(
                    res, pat[:, :, :].rearrange("p g w -> p (g w)"),
                    xres, mybir.AluOpType.add)
                eng.dma_start(out[:, g * 512 : (g + 1) * 512], res)

    return nc


def _prep_core_inputs(core, x, Wq, bq, Wk, bk, Wv, bv, gamma):
    b = core // 2
    p = core % 2
    g = float(np.asarray(gamma).reshape(-1)[0])
    qsl = slice(16 * p, 16 * p + 16)
    vsl = slice(128 * p, 128 * p + 128)

    import ml_dtypes
    bf = ml_dtypes.bfloat16

    wqk = np.zeros((C, 32), np.float32)
    wqk[:, 0:16] = Wq[qsl].T       # q head even(8) | q head odd(8)
    wqk[:, 16:32] = Wk[qsl].T
    wqk = wqk.astype(bf)
    qkb = np.concatenate([bq[qsl], bk[qsl]]).reshape(32, 1).astype(np.float32)

    wv_eff = (g * Wv[vsl]).astype(np.float32)     # [128, 256]
    bv_eff = (g * bv[vsl]).astype(np.float32)
    wvt = np.zeros((C, 130), np.float32)
    wvt[:, 0:64] = wv_eff[0:64].T
    wvt[:, 65:129] = wv_eff[64:128].T
    wvt = wvt.astype(bf)
    vbias = np.zeros((1, 130), np.float32)
    vbias[0, 0:64] = bv_eff[0:64]
    vbias[0, 64] = 1.0
    vbias[0, 65:129] = bv_eff[64:128]
    vbias[0, 129] = 1.0
    vbias = vbias.astype(bf)

    return {
        "x_in": np.ascontiguousarray(x[b].reshape(C, PIX), np.float32),
        "x_res": np.ascontiguousarray(x[b, vsl].reshape(128, PIX), np.float32),
        "wqkT": wqk,
        "qk_bias": qkb,
        "wvT": wvt,
        "vbias_row": vbias,
    }


_NC_CACHE = None


def _get_nc():
    global _NC_CACHE
    if _NC_CACHE is None:
        nc = build_program()
        nc.compile()
        _NC_CACHE = nc
    return _NC_CACHE


def kernel(x, Wq, bq, Wk, bk, Wv, bv, gamma, _trace=False, _trace_kwargs=None):
    from concourse.bass_utils import run_bass_kernel_spmd

    nc = _get_nc()
    in_maps = [
        _prep_core_inputs(core, x, Wq, bq, Wk, bk, Wv, bv, gamma)
        for core in range(NCORES)
    ]
    res = run_bass_kernel_spmd(
        nc, in_maps, list(range(NCORES)), trace=_trace,
        **(_trace_kwargs or {}),
    )
    outp = np.empty((B, C, H, W), np.float32)
    for core in range(NCORES):
        b, p = core // 2, core % 2
        outp[b, 128 * p : 128 * p + 128] = (
            res.results[core]["out"].reshape(128, H, W)
        )
    if _trace:
        kernel.last_results = res
    return outp



# revision 7
# speedup vs baseline: 1.0541x; 1.0541x over previous
"""CrissCrossAttention Trainium2 kernel (8 NeuronCores, data-parallel).

Problem: B=4, C=256, H=W=128, 4 heads. Per head: cq=8 q/k channels, cv=64
v channels. Row attention (over W per row) + column attention (over H per
column), outputs added with the CCNet spatial-transpose quirk, then
out = gamma*attn + x.

Sharding: 16 (batch, head) pairs over 8 cores -> each core handles
batch b = core//2 and head pair p = core%2 (global heads 2p, 2p+1).
Each core reads x[b] (all 256 channels, needed by the projections) and
produces output channels [128p : 128p+128] of batch b.

Host-side prep per core: x is converted to bf16 and channel-reordered so
the residual slice (the 128 output channels) is rows 0-127; weight rows
are permuted identically. Output is bf16 (upcast on host).

Core-local pipeline (pixels indexed pix = h*128 + w):
  - qk projection -> flat row-major fr[32, h*128+w] bf16; col-major
    fc[32, w*128+h] via incremental DVE permute. Bias fused into the
    PSUM evacuation.
  - band-packed operand stores for the PE (matmul operands must start at
    32-aligned partitions): q/k value for row h lives at partition
    32*(h%4)+c -> the 4 rows of a group occupy distinct PE row-groups and
    their K=8 energy matmuls run concurrently via tile_position (each into
    its own PSUM bank). Built with SBUF->SBUF DMAs: q/k issued
    incrementally every 8 chunks during the projection loop; qc/kc after
    fc completes, spread over 4 issuing engines.
  - vT projection (pixel-major): vT[128w, 128h, 130c] bf16, channels =
    [64 head0 | 1 | 64 head1 | 1] with ones channels for the softmax
    denominator. Evacuation = DVE add of replicated bias (no PE bias
    matmul).
  - vTc[h, w, c] = spatial transpose of vT via per-channel PE transposes.
  - Per group g (4 rows), head hh: eT[v,w] = k^T q (PE, K=8, 4 rows
    concurrent, both hh into one 4-bank PSUM tile at free offset 128*hh);
    pT = exp(eT) (ACT); po2[w, j, hh, 0:65] = pT.T @ vT slice (65th col =
    denominator); one reciprocal + one multiply per (g, dir) covering
    both hh -> t[w, j, hh, c].
  - Column attention identical using qc/kc stores and vTc. CCNet combine:
    attn_un = t_row + t_col elementwise (GpSimd); PE-transpose to
    channel-major; bf16 residual add (DVE); DMA out bf16.
"""

import os
import numpy as np
from contextlib import ExitStack

import concourse.bass as bass
import concourse.bacc as bacc
import concourse.tile as tile
from concourse import mybir
from concourse.masks import make_identity

F32 = mybir.dt.float32
BF16 = mybir.dt.bfloat16

B, C, H, W = 4, 256, 128, 128
PIX = H * W            # 16384
CV = 64                # v channels per head
NCORES = 8
G = 4                  # rows per attention group (= PE row-group packing)
NG = H // G            # 32 groups


def build_program():
    nc = bacc.Bacc("TRN2", target_bir_lowering=False, debug=False,
                   num_devices=NCORES)

    x_in = nc.dram_tensor("x_in", [C, PIX], BF16, kind="ExternalInput")
    wqkT = nc.dram_tensor("wqkT", [C, 32], BF16, kind="ExternalInput")
    qk_bias = nc.dram_tensor("qk_bias", [32, 1], F32, kind="ExternalInput")
    wvT = nc.dram_tensor("wvT", [C, 130], BF16, kind="ExternalInput")
    vbias_full = nc.dram_tensor("vbias_full", [128, 130], BF16,
                                kind="ExternalInput")
    out = nc.dram_tensor("out", [128, PIX], BF16, kind="ExternalOutput")

    with tile.TileContext(nc) as tc, ExitStack() as ctx:
        consts = ctx.enter_context(tc.tile_pool(name="consts", bufs=1))
        persist = ctx.enter_context(tc.tile_pool(name="persist", bufs=1))

        # constants / weights
        wqa = consts.tile([128, 32], BF16, tag="wqa")
        wqb = consts.tile([128, 32], BF16, tag="wqb")
        nc.sync.dma_start(wqa, wqkT[0:128, :])
        nc.sync.dma_start(wqb, wqkT[128:256, :])
        wva = consts.tile([128, 130], BF16, tag="wva")
        wvb = consts.tile([128, 130], BF16, tag="wvb")
        nc.sync.dma_start(wva, wvT[0:128, :])
        nc.sync.dma_start(wvb, wvT[128:256, :])
        qkb = consts.tile([32, 1], F32, tag="qkb")
        nc.sync.dma_start(qkb, qk_bias[:, :])
        vbias = consts.tile([128, 1, 130], BF16, tag="vbias")
        nc.sync.dma_start(vbias[:, 0, :], vbias_full[:, :])
        identb = consts.tile([128, 128], BF16, tag="identb")
        make_identity(nc, identb)

        # persistent activations
        # band-packed operand stores: partition 32*(h%4)+c, c<8
        q_sb = persist.tile([128, 2, H // 4, W], BF16, tag="q")    # 16 KiB
        k_sb = persist.tile([128, 2, H // 4, W], BF16, tag="k")    # 16 KiB
        qc_sb = persist.tile([128, 2, W // 4, H], BF16, tag="qc")  # 16 KiB
        kc_sb = persist.tile([128, 2, W // 4, H], BF16, tag="kc")  # 16 KiB
        # pixel-major value stores, channel innermost
        vT_sb = persist.tile([128, H, 130], BF16, tag="vT")        # 32.5 KiB
        vTc_sb = persist.tile([128, W, 130], BF16, tag="vTc")      # 32.5 KiB

        # ---------------- Phase B: projections ----------------
        with (
            tc.tile_pool(name="qkflat", bufs=1) as flatpool,
            tc.tile_pool(name="xchunk", bufs=2) as xpool,
            tc.tile_pool(name="pq", bufs=2, space="PSUM") as pqpool,
            tc.tile_pool(name="pv", bufs=4, space="PSUM") as pvpool,
        ):
            fr = flatpool.tile([32, PIX], BF16, tag="fr")  # [c, h*128+w]
            fc = flatpool.tile([32, PIX], BF16, tag="fc")  # [c, w*128+h]

            def bandpack_rows(dst_q, dst_k, src4, hb0, nhb, engs):
                # src4: [c, b, hb, w/h] view of fr or fc
                ei = 0
                for bb in range(4):
                    for hh in range(2):
                        eng = engs[ei % len(engs)]
                        ei += 1
                        eng.dma_start(
                            dst_q[32 * bb : 32 * bb + 8, hh, hb0 : hb0 + nhb, :],
                            src4[8 * hh : 8 * hh + 8, bb, hb0 : hb0 + nhb, :])
                        eng.dma_start(
                            dst_k[32 * bb : 32 * bb + 8, hh, hb0 : hb0 + nhb, :],
                            src4[16 + 8 * hh : 24 + 8 * hh, bb, hb0 : hb0 + nhb, :])

            src_r = fr[:, :].rearrange("c (hb b w) -> c b hb w", b=4, w=W)
            src_c = fc[:, :].rearrange("c (wb b h) -> c b wb h", b=4, h=H)

            CHUNK = 512  # pixels per chunk = 4 rows
            NCH = PIX // CHUNK
            for chi in range(NCH):
                c0 = chi * CHUNK
                r0 = c0 // 128
                eng = nc.sync if chi % 2 == 0 else nc.scalar
                xab = xpool.tile([128, CHUNK], BF16, tag="xab")
                xbb = xpool.tile([128, CHUNK], BF16, tag="xbb")
                eng.dma_start(xab, x_in[0:128, c0 : c0 + CHUNK])
                eng.dma_start(xbb, x_in[128:256, c0 : c0 + CHUNK])
                xav = xab[:, :].rearrange("p (r w) -> p r w", w=128)
                xbv = xbb[:, :].rearrange("p (r w) -> p r w", w=128)

                # qk projection, row-pixel order
                pq = pqpool.tile([32, 512], F32, tag="pq")
                nc.tensor.matmul(pq, wqa, xab[:, :], start=True, stop=False)
                nc.tensor.matmul(pq, wqb, xbb[:, :], start=False, stop=True)
                nc.vector.tensor_scalar_add(fr[:, c0 : c0 + CHUNK], pq, qkb)

                # vT projection: 2 rows per PSUM half-bank tile; bias is
                # added at evacuation (DVE), not via a PE matmul
                for s2 in range(2):
                    pv = pvpool.tile([128, 2, 130], F32, tag="pv")
                    for s3 in range(2):
                        srow = 2 * s2 + s3
                        nc.tensor.matmul(pv[:, s3, :], xav[:, srow, :], wva,
                                         start=(s3 == 0), stop=False,
                                         skip_group_check=True)
                        nc.tensor.matmul(pv[:, s3, :], xbv[:, srow, :], wvb,
                                         start=False, stop=(s3 == 1),
                                         skip_group_check=True)
                    nc.vector.tensor_tensor(
                        vT_sb[:, r0 + 2 * s2 : r0 + 2 * s2 + 2, :], pv,
                        vbias.to_broadcast((128, 2, 130)),
                        mybir.AluOpType.add)

                # col-major flat store slices: fc[:, :, h-slice] only needs
                # fr rows h-slice -> overlap the permute with projection
                if chi % 8 == 7:
                    hs = (chi // 8) * 32
                    frv = fr[:, :].rearrange("c (h w) -> c w h", w=W)
                    fcv = fc[:, :].rearrange("c (w h) -> c w h", h=H)
                    nc.vector.tensor_copy(fcv[:, :, hs : hs + 32],
                                          frv[:, :, hs : hs + 32])
                    # row-direction band-pack for the 8 groups just done
                    bandpack_rows(q_sb, k_sb, src_r, chi - 7, 8,
                                  [nc.sync, nc.scalar])

            # column-direction band-pack (needs the full fc); spread over
            # four issuing engines to hit all DMA queues
            bandpack_rows(qc_sb, kc_sb, src_c, 0, 32,
                          [nc.sync, nc.scalar, nc.gpsimd])

        # ---------------- Phase B2: vTc via PE transposes ----------------
        # vT[w, h, c] -> vTc[h, w, c]; per channel, batched 4 per bank.
        with tc.tile_pool(name="ptr", bufs=2, space="PSUM") as ptrpool:
            for cb in range(33):
                nch = min(4, 130 - cb * 4)
                ptr = ptrpool.tile([128, 4, 128], BF16, tag="ptr")
                for cj in range(nch):
                    cch = cb * 4 + cj
                    nc.tensor.matmul(ptr[:, cj, :], vT_sb[:, :, cch], identb,
                                     start=True, stop=True, is_transpose=True)
                nc.vector.tensor_copy(
                    vTc_sb[:, :, cb * 4 : cb * 4 + nch],
                    ptr[:, 0:nch, :].rearrange("p c w -> p w c"))

        # ---------------- Phase C: attention ----------------
        with (
            tc.tile_pool(name="pe", bufs=1, space="PSUM") as pepool,
            tc.tile_pool(name="po", bufs=1, space="PSUM") as popool,
            tc.tile_pool(name="pat", bufs=2, space="PSUM") as patpool,
            tc.tile_pool(name="pt", bufs=3) as ptpool,
            tc.tile_pool(name="tt", bufs=3) as tpool,
            tc.tile_pool(name="au", bufs=2) as aupool,
            tc.tile_pool(name="rc", bufs=4) as rcpool,
            tc.tile_pool(name="io", bufs=3) as iopool,
        ):
            for g in range(NG):
                t_dir = []
                for d in range(2):  # 0 = row attention, 1 = column attention
                    qs = q_sb if d == 0 else qc_sb
                    ks = k_sb if d == 0 else kc_sb
                    vs = vT_sb if d == 0 else vTc_sb
                    til = tpool.tile([128, G, 2, CV], BF16, tag="t")
                    # energies for both heads into one 4-bank PSUM tile:
                    # head hh at free offset 128*hh of bank j
                    pe = pepool.tile([128, G, 512], F32, tag="pe")
                    pT = []
                    for hh in range(2):
                        for j in range(G):
                            nc.tensor.matmul(
                                pe[:, j, 128 * hh : 128 * hh + 128],
                                ks[32 * j : 32 * j + 8, hh, g, :],
                                qs[32 * j : 32 * j + 8, hh, g, :],
                                start=True, stop=True,
                                tile_position=(32 * j, 0),
                                skip_group_check=True,
                            )
                        pTh = ptpool.tile([128, G, 128], BF16, tag="pt")
                        nc.scalar.activation(
                            pTh, pe[:, :, 128 * hh : 128 * hh + 128],
                            mybir.ActivationFunctionType.Exp)
                        pT.append(pTh)
                    # one PSUM bank per head; 512B-aligned 65-wide slices so
                    # no matmul output straddles a bank boundary
                    po = popool.tile([128, 2, G, 128], F32, tag="po")
                    for hh in range(2):
                        for j in range(G):
                            i = g * G + j
                            nc.tensor.matmul(
                                po[:, hh, j, 0:65], pT[hh][:, j, :],
                                vs[:, i, 65 * hh : 65 * hh + 65],
                                start=True, stop=True,
                                skip_group_check=True,
                            )
                    pov = po[:, :, :, :].rearrange("p hh j c -> p j hh c")
                    rec = rcpool.tile([128, G, 2, 1], F32, tag="rc")
                    nc.vector.reciprocal(rec, pov[:, :, :, 64:65])
                    nc.vector.tensor_tensor(
                        til, pov[:, :, :, 0:64],
                        rec.to_broadcast((128, G, 2, CV)),
                        mybir.AluOpType.mult,
                    )
                    t_dir.append(til)
                au = aupool.tile([128, G, 128], BF16, tag="au")
                nc.gpsimd.tensor_tensor(au, t_dir[0][:, :, :, :],
                                        t_dir[1][:, :, :, :],
                                        mybir.AluOpType.add)
                pat = patpool.tile([128, G, 128], BF16, tag="pat")
                for j in range(G):
                    nc.tensor.matmul(pat[:, j, :], au[:, j, :], identb,
                                     start=True, stop=True, is_transpose=True)
                eng = nc.sync if g % 2 == 0 else nc.scalar
                xres = iopool.tile([128, G * 128], BF16, tag="xres")
                eng.dma_start(xres, x_in[0:128, g * 512 : (g + 1) * 512])
                res = iopool.tile([128, G * 128], BF16, tag="res")
                nc.vector.tensor_tensor(
                    res, pat[:, :, :].rearrange("p g w -> p (g w)"),
                    xres, mybir.AluOpType.add)
                eng.dma_start(out[:, g * 512 : (g + 1) * 512], res)

    return nc


def _prep_core_inputs(core, x, Wq, bq, Wk, bk, Wv, bv, gamma):
    b = core // 2
    p = core % 2
    g = float(np.asarray(gamma).reshape(-1)[0])
    qsl = slice(16 * p, 16 * p + 16)
    vsl = slice(128 * p, 128 * p + 128)

    import ml_dtypes
    bf = ml_dtypes.bfloat16

    # channel permutation: residual (output) channels first
    perm = np.concatenate([np.arange(128 * p, 128 * p + 128),
                           np.arange(128 * (1 - p), 128 * (1 - p) + 128)])

    wqk = np.zeros((C, 32), np.float32)
    wqk[:, 0:16] = Wq[qsl].T       # q head even(8) | q head odd(8)
    wqk[:, 16:32] = Wk[qsl].T
    wqk = wqk[perm].astype(bf)
    qkb = np.concatenate([bq[qsl], bk[qsl]]).reshape(32, 1).astype(np.float32)

    wv_eff = (g * Wv[vsl]).astype(np.float32)     # [128, 256]
    bv_eff = (g * bv[vsl]).astype(np.float32)
    wvt = np.zeros((C, 130), np.float32)
    wvt[:, 0:64] = wv_eff[0:64].T
    wvt[:, 65:129] = wv_eff[64:128].T
    wvt = wvt[perm].astype(bf)
    vbias = np.zeros((1, 130), np.float32)
    vbias[0, 0:64] = bv_eff[0:64]
    vbias[0, 64] = 1.0
    vbias[0, 65:129] = bv_eff[64:128]
    vbias[0, 129] = 1.0
    vbias_full = np.broadcast_to(vbias, (128, 130)).astype(bf)

    x2 = x[b].reshape(C, PIX)[perm]
    return {
        "x_in": np.ascontiguousarray(x2).astype(bf),
        "wqkT": wqk,
        "qk_bias": qkb,
        "wvT": wvt,
        "vbias_full": np.ascontiguousarray(vbias_full),
    }


_NC_CACHE = None


def _get_nc():
    global _NC_CACHE
    if _NC_CACHE is None:
        nc = build_program()
        nc.compile()
        _NC_CACHE = nc
    return _NC_CACHE


def kernel(x, Wq, bq, Wk, bk, Wv, bv, gamma, _trace=False, _trace_kwargs=None):
    from concourse.bass_utils import run_bass_kernel_spmd

    nc = _get_nc()
    in_maps = [
        _prep_core_inputs(core, x, Wq, bq, Wk, bk, Wv, bv, gamma)
        for core in range(NCORES)
    ]
    res = run_bass_kernel_spmd(
        nc, in_maps, list(range(NCORES)), trace=_trace,
        **(_trace_kwargs or {}),
    )
    outp = np.empty((B, C, H, W), np.float32)
    for core in range(NCORES):
        b, p = core // 2, core % 2
        outp[b, 128 * p : 128 * p + 128] = (
            res.results[core]["out"].astype(np.float32).reshape(128, H, W)
        )
    if _trace:
        kernel.last_results = res
    return outp


# revision 8
# speedup vs baseline: 1.5768x; 1.4959x over previous
"""CrissCrossAttention Trainium2 kernel (8 NeuronCores, data-parallel).

Problem: B=4, C=256, H=W=128, 4 heads. Per head: cq=8 q/k channels, cv=64
v channels. Row attention (over W per row) + column attention (over H per
column), outputs added with the CCNet spatial-transpose quirk, then
out = gamma*attn + x.

Sharding: 16 (batch, head) pairs over 8 cores -> each core handles
batch b = core//2 and head pair p = core%2 (global heads 2p, 2p+1).
Each core reads x[b] (all 256 channels, needed by the projections) and
produces output channels [128p : 128p+128] of batch b.

Host-side prep per core: x is bf16, channel-reordered so the residual
slice is rows 0-127 of x_in; weight rows permuted identically. A second
pixel-major copy x_pix[w, h*128+ch] feeds the residual add. Output is
bf16 pixel-major [w, h*128+ch]; host transposes/upcasts.

Core-local pipeline (pixels indexed pix = h*128 + w):
  - qk projection -> flat row-major fr[32, h*128+w] bf16; col-major
    fc[32, w*128+h] via incremental DVE permute. Bias fused into the
    PSUM evacuation.
  - band-packed operand stores for the PE (matmul operands must start at
    32-aligned partitions): q/k value for row h lives at partition
    32*(h%4)+c -> the 4 rows of a group occupy distinct PE row-groups and
    their K=8 energy matmuls run concurrently via tile_position (each into
    its own PSUM bank). Built with SBUF->SBUF DMAs: q/k issued
    incrementally during the projection loop; qc/kc after fc completes,
    spread over 3 issuing engines.
  - vT projection (pixel-major): vT[128w, 128h, 130c] bf16, channels =
    [64 head0 | 1 | 64 head1 | 1] with ones channels for the softmax
    denominator. Evacuation = DVE add of replicated bias.
  - vTc[h, w, c] = spatial transpose of vT via per-channel PE transposes.
  - Attention is software-pipelined one group deep so the PE never waits
    on the ACT exp: per group g, issue all 16 energy matmuls (dirs x
    heads x 4 rows, 4-way concurrent into pe's 4 banks), the two EXPs
    (one per dir, covering both heads), then the PREVIOUS group's 16
    value matmuls po (po's own 4 banks), reciprocal+multiply (DVE),
    CCNet combine t_row+t_col (GpSimd), residual add (DVE, pixel-major),
    DMA out. PSUM = pe 4 banks + po 4 banks, exact fit.
"""

import os
import numpy as np
from contextlib import ExitStack

import concourse.bass as bass
import concourse.bacc as bacc
import concourse.tile as tile
from concourse import mybir
from concourse.masks import make_identity

F32 = mybir.dt.float32
BF16 = mybir.dt.bfloat16

B, C, H, W = 4, 256, 128, 128
PIX = H * W            # 16384
CV = 64                # v channels per head
NCORES = 8
G = 4                  # rows per attention group (= PE row-group packing)
NG = H // G            # 32 groups


def build_program():
    nc = bacc.Bacc("TRN2", target_bir_lowering=False, debug=False,
                   num_devices=NCORES)

    x_in = nc.dram_tensor("x_in", [C, PIX], BF16, kind="ExternalInput")
    x_pix = nc.dram_tensor("x_pix", [W, H * 128], BF16, kind="ExternalInput")
    wqkT = nc.dram_tensor("wqkT", [C, 32], BF16, kind="ExternalInput")
    qk_bias = nc.dram_tensor("qk_bias", [32, 1], F32, kind="ExternalInput")
    wvT = nc.dram_tensor("wvT", [C, 130], BF16, kind="ExternalInput")
    vbias_full = nc.dram_tensor("vbias_full", [128, 130], BF16,
                                kind="ExternalInput")
    out = nc.dram_tensor("out", [W, H * 128], BF16, kind="ExternalOutput")

    with tile.TileContext(nc) as tc, ExitStack() as ctx:
        consts = ctx.enter_context(tc.tile_pool(name="consts", bufs=1))
        persist = ctx.enter_context(tc.tile_pool(name="persist", bufs=1))

        # constants / weights
        wqa = consts.tile([128, 32], BF16, tag="wqa")
        wqb = consts.tile([128, 32], BF16, tag="wqb")
        nc.sync.dma_start(wqa, wqkT[0:128, :])
        nc.sync.dma_start(wqb, wqkT[128:256, :])
        wva = consts.tile([128, 130], BF16, tag="wva")
        wvb = consts.tile([128, 130], BF16, tag="wvb")
        nc.sync.dma_start(wva, wvT[0:128, :])
        nc.sync.dma_start(wvb, wvT[128:256, :])
        qkb = consts.tile([32, 1], F32, tag="qkb")
        nc.sync.dma_start(qkb, qk_bias[:, :])
        vbias = consts.tile([128, 1, 130], BF16, tag="vbias")
        nc.sync.dma_start(vbias[:, 0, :], vbias_full[:, :])
        identb = consts.tile([128, 128], BF16, tag="identb")
        make_identity(nc, identb)

        # persistent activations
        # band-packed operand stores: partition 32*(h%4)+c, c<8
        q_sb = persist.tile([128, 2, H // 4, W], BF16, tag="q")    # 16 KiB
        k_sb = persist.tile([128, 2, H // 4, W], BF16, tag="k")    # 16 KiB
        qc_sb = persist.tile([128, 2, W // 4, H], BF16, tag="qc")  # 16 KiB
        kc_sb = persist.tile([128, 2, W // 4, H], BF16, tag="kc")  # 16 KiB
        # pixel-major value stores, channel innermost
        vT_sb = persist.tile([128, H, 130], BF16, tag="vT")        # 32.5 KiB
        vTc_sb = persist.tile([128, W, 130], BF16, tag="vTc")      # 32.5 KiB

        # ---------------- Phase B: projections ----------------
        with (
            tc.tile_pool(name="qkflat", bufs=1) as flatpool,
            tc.tile_pool(name="xchunk", bufs=2) as xpool,
            tc.tile_pool(name="pq", bufs=2, space="PSUM") as pqpool,
            tc.tile_pool(name="pv", bufs=4, space="PSUM") as pvpool,
        ):
            fr = flatpool.tile([32, PIX], BF16, tag="fr")  # [c, h*128+w]
            fc = flatpool.tile([32, PIX], BF16, tag="fc")  # [c, w*128+h]

            def bandpack_rows(dst_q, dst_k, src4, hb0, nhb, engs):
                # src4: [c, b, hb, w/h] view of fr or fc
                ei = 0
                for bb in range(4):
                    for hh in range(2):
                        eng = engs[ei % len(engs)]
                        ei += 1
                        eng.dma_start(
                            dst_q[32 * bb : 32 * bb + 8, hh, hb0 : hb0 + nhb, :],
                            src4[8 * hh : 8 * hh + 8, bb, hb0 : hb0 + nhb, :])
                        eng.dma_start(
                            dst_k[32 * bb : 32 * bb + 8, hh, hb0 : hb0 + nhb, :],
                            src4[16 + 8 * hh : 24 + 8 * hh, bb, hb0 : hb0 + nhb, :])

            src_r = fr[:, :].rearrange("c (hb b w) -> c b hb w", b=4, w=W)
            src_c = fc[:, :].rearrange("c (wb b h) -> c b wb h", b=4, h=H)

            CHUNK = 1024  # pixels per chunk = 8 rows
            NCH = PIX // CHUNK
            for chi in range(NCH):
                c0 = chi * CHUNK
                r0 = c0 // 128
                eng = nc.sync if chi % 2 == 0 else nc.scalar
                xab = xpool.tile([128, CHUNK], BF16, tag="xab")
                xbb = xpool.tile([128, CHUNK], BF16, tag="xbb")
                eng.dma_start(xab, x_in[0:128, c0 : c0 + CHUNK])
                eng.dma_start(xbb, x_in[128:256, c0 : c0 + CHUNK])
                xav = xab[:, :].rearrange("p (r w) -> p r w", w=128)
                xbv = xbb[:, :].rearrange("p (r w) -> p r w", w=128)

                # qk projection, row-pixel order (matmul out <= 1 bank)
                pq = pqpool.tile([32, 2, 512], F32, tag="pq")
                for s in range(2):
                    nc.tensor.matmul(pq[:, s, :], wqa,
                                     xab[:, 512 * s : 512 * s + 512],
                                     start=True, stop=False,
                                     skip_group_check=True)
                    nc.tensor.matmul(pq[:, s, :], wqb,
                                     xbb[:, 512 * s : 512 * s + 512],
                                     start=False, stop=True,
                                     skip_group_check=True)
                nc.vector.tensor_scalar_add(
                    fr[:, c0 : c0 + CHUNK],
                    pq[:, :, :].rearrange("p s w -> p (s w)"), qkb)

                # vT projection: 2 rows per PSUM half-bank tile; bias is
                # added at evacuation (DVE), not via a PE matmul
                for s2 in range(4):
                    pv = pvpool.tile([128, 2, 130], F32, tag="pv")
                    for s3 in range(2):
                        srow = 2 * s2 + s3
                        nc.tensor.matmul(pv[:, s3, :], xav[:, srow, :], wva,
                                         start=(s3 == 0), stop=False,
                                         skip_group_check=True)
                        nc.tensor.matmul(pv[:, s3, :], xbv[:, srow, :], wvb,
                                         start=False, stop=(s3 == 1),
                                         skip_group_check=True)
                    nc.vector.tensor_tensor(
                        vT_sb[:, r0 + 2 * s2 : r0 + 2 * s2 + 2, :], pv,
                        vbias.to_broadcast((128, 2, 130)),
                        mybir.AluOpType.add)

                # col-major flat store slices: fc[:, :, h-slice] only needs
                # fr rows h-slice -> overlap the permute with projection
                if chi % 4 == 3:
                    hs = (chi // 4) * 32
                    frv = fr[:, :].rearrange("c (h w) -> c w h", w=W)
                    fcv = fc[:, :].rearrange("c (w h) -> c w h", h=H)
                    nc.vector.tensor_copy(fcv[:, :, hs : hs + 32],
                                          frv[:, :, hs : hs + 32])
                    # row-direction band-pack for the 8 groups just done
                    bandpack_rows(q_sb, k_sb, src_r, (chi // 4) * 8, 8,
                                  [nc.sync, nc.scalar])

            # column-direction band-pack (needs the full fc); spread over
            # three issuing engines
            bandpack_rows(qc_sb, kc_sb, src_c, 0, 32,
                          [nc.sync, nc.scalar, nc.gpsimd])

        # ---------------- Phase B2: vTc via PE transposes ----------------
        # vT[w, h, c] -> vTc[h, w, c]; per channel, batched 4 per bank.
        with tc.tile_pool(name="ptr", bufs=2, space="PSUM") as ptrpool:
            for cb in range(33):
                nch = min(4, 130 - cb * 4)
                ptr = ptrpool.tile([128, 4, 128], BF16, tag="ptr")
                for cj in range(nch):
                    cch = cb * 4 + cj
                    nc.tensor.matmul(ptr[:, cj, :], vT_sb[:, :, cch], identb,
                                     start=True, stop=True, is_transpose=True)
                nc.vector.tensor_copy(
                    vTc_sb[:, :, cb * 4 : cb * 4 + nch],
                    ptr[:, 0:nch, :].rearrange("p c w -> p w c"))

        # ---------------- Phase C: attention (pipelined) ----------------
        with (
            tc.tile_pool(name="pe", bufs=1, space="PSUM") as pepool,
            tc.tile_pool(name="po", bufs=1, space="PSUM") as popool,
            tc.tile_pool(name="pt", bufs=4) as ptpool,
            tc.tile_pool(name="tt", bufs=3) as tpool,
            tc.tile_pool(name="au", bufs=2) as aupool,
            tc.tile_pool(name="rc", bufs=4) as rcpool,
            tc.tile_pool(name="io", bufs=3) as iopool,
        ):
            def energies(g):
                # all 16 energy matmuls into pe's 4 banks (bank = row j),
                # then one EXP per direction covering both heads
                pe = pepool.tile([128, G, 512], F32, tag="pe")
                for d in range(2):
                    qs = q_sb if d == 0 else qc_sb
                    ks = k_sb if d == 0 else kc_sb
                    for hh in range(2):
                        for j in range(G):
                            nc.tensor.matmul(
                                pe[:, j, 256 * d + 128 * hh :
                                   256 * d + 128 * hh + 128],
                                ks[32 * j : 32 * j + 8, hh, g, :],
                                qs[32 * j : 32 * j + 8, hh, g, :],
                                start=True, stop=True,
                                tile_position=(32 * j, 0),
                                skip_group_check=True,
                            )
                pTs = []
                for d in range(2):
                    pT = ptpool.tile([128, G, 256], BF16, tag="pt")
                    nc.scalar.activation(
                        pT, pe[:, :, 256 * d : 256 * d + 256],
                        mybir.ActivationFunctionType.Exp)
                    pTs.append(pT)
                return pTs

            def attend(g, pTs):
                po = popool.tile([128, 2, 2, G, 128], F32, tag="po")
                for d in range(2):
                    vs = vT_sb if d == 0 else vTc_sb
                    for hh in range(2):
                        for j in range(G):
                            i = g * G + j
                            nc.tensor.matmul(
                                po[:, d, hh, j, 0:65],
                                pTs[d][:, j, 128 * hh : 128 * hh + 128],
                                vs[:, i, 65 * hh : 65 * hh + 65],
                                start=True, stop=True,
                                skip_group_check=True,
                            )
                tils = []
                for d in range(2):
                    til = tpool.tile([128, G, 2, CV], BF16, tag="t")
                    pov = po[:, d, :, :, :].rearrange("p hh j c -> p j hh c")
                    rec = rcpool.tile([128, G, 2, 1], F32, tag="rc")
                    nc.vector.reciprocal(rec, pov[:, :, :, 64:65])
                    nc.vector.tensor_tensor(
                        til, pov[:, :, :, 0:64],
                        rec.to_broadcast((128, G, 2, CV)),
                        mybir.AluOpType.mult,
                    )
                    tils.append(til)
                au = aupool.tile([128, G, 128], BF16, tag="au")
                nc.gpsimd.tensor_tensor(au, tils[0][:, :, :, :],
                                        tils[1][:, :, :, :],
                                        mybir.AluOpType.add)
                eng = nc.sync if g % 2 == 0 else nc.scalar
                xpg = iopool.tile([128, G * 128], BF16, tag="xpg")
                eng.dma_start(xpg, x_pix[:, g * 512 : (g + 1) * 512])
                res = iopool.tile([128, G * 128], BF16, tag="res")
                nc.vector.tensor_tensor(
                    res, au[:, :, :].rearrange("p g w -> p (g w)"),
                    xpg, mybir.AluOpType.add)
                eng.dma_start(out[:, g * 512 : (g + 1) * 512], res)

            prev = energies(0)
            for g in range(1, NG):
                cur = energies(g)
                attend(g - 1, prev)
                prev = cur
            attend(NG - 1, prev)

    return nc


def _prep_core_inputs(core, x, Wq, bq, Wk, bk, Wv, bv, gamma):
    b = core // 2
    p = core % 2
    g = float(np.asarray(gamma).reshape(-1)[0])
    qsl = slice(16 * p, 16 * p + 16)
    vsl = slice(128 * p, 128 * p + 128)

    import ml_dtypes
    bf = ml_dtypes.bfloat16

    # channel permutation: residual (output) channels first
    perm = np.concatenate([np.arange(128 * p, 128 * p + 128),
                           np.arange(128 * (1 - p), 128 * (1 - p) + 128)])

    wqk = np.zeros((C, 32), np.float32)
    wqk[:, 0:16] = Wq[qsl].T       # q head even(8) | q head odd(8)
    wqk[:, 16:32] = Wk[qsl].T
    wqk = wqk[perm].astype(bf)
    qkb = np.concatenate([bq[qsl], bk[qsl]]).reshape(32, 1).astype(np.float32)

    wv_eff = (g * Wv[vsl]).astype(np.float32)     # [128, 256]
    bv_eff = (g * bv[vsl]).astype(np.float32)
    wvt = np.zeros((C, 130), np.float32)
    wvt[:, 0:64] = wv_eff[0:64].T
    wvt[:, 65:129] = wv_eff[64:128].T
    wvt = wvt[perm].astype(bf)
    vbias = np.zeros((1, 130), np.float32)
    vbias[0, 0:64] = bv_eff[0:64]
    vbias[0, 64] = 1.0
    vbias[0, 65:129] = bv_eff[64:128]
    vbias[0, 129] = 1.0
    vbias_full = np.broadcast_to(vbias, (128, 130)).astype(bf)

    x2 = x[b].reshape(C, PIX)[perm]
    # pixel-major residual: [w][h][ch] so per-group loads are contiguous
    xpix = np.ascontiguousarray(
        x[b, vsl].transpose(2, 1, 0)).reshape(W, H * 128)
    return {
        "x_in": np.ascontiguousarray(x2).astype(bf),
        "x_pix": xpix.astype(bf),
        "wqkT": wqk,
        "qk_bias": qkb,
        "wvT": wvt,
        "vbias_full": np.ascontiguousarray(vbias_full),
    }


_NC_CACHE = None


def _get_nc():
    global _NC_CACHE
    if _NC_CACHE is None:
        nc = build_program()
        nc.compile()
        _NC_CACHE = nc
    return _NC_CACHE


def kernel(x, Wq, bq, Wk, bk, Wv, bv, gamma, _trace=False, _trace_kwargs=None):
    from concourse.bass_utils import run_bass_kernel_spmd

    nc = _get_nc()
    in_maps = [
        _prep_core_inputs(core, x, Wq, bq, Wk, bk, Wv, bv, gamma)
        for core in range(NCORES)
    ]
    res = run_bass_kernel_spmd(
        nc, in_maps, list(range(NCORES)), trace=_trace,
        **(_trace_kwargs or {}),
    )
    outp = np.empty((B, C, H, W), np.float32)
    for core in range(NCORES):
        b, p = core // 2, core % 2
        o = res.results[core]["out"].astype(np.float32)
        # out[w, h*128+ch] -> [ch, h, w]
        outp[b, 128 * p : 128 * p + 128] = (
            o.reshape(W, H, 128).transpose(2, 1, 0)
        )
    if _trace:
        kernel.last_results = res
    return outp
